# revision 33
# baseline (speedup 1.0000x reference)
"""Trainium2 Bass kernel: GNN message passing (gather + weighted segment-sum) + 3-layer MLP.

Strategy (8 NeuronCores, SPMD, no collectives):
  - Destination nodes are sharded 12500/core; the bf16 feature table is
    pair-packed to [50000, 128] (two 64-wide rows per 256B row) and
    replicated into every core's HBM.
  - Host sorts each core's edges by (dst-tile, src-segment, src-parity,
    src) and pads every cell to a chunk structure that is *uniform across
    cores*, so one compiled NEFF serves all 8 cores (only tensor data
    differs).
  - The gather runs as 1024-index single_packet dma_gather calls spread
    round-robin over 4 SWDGE queues (descriptor generation is parallel
    across Q7 core pairs — the single-queue serial gather was the original
    bottleneck); src-sorted indices give HBM row locality.
  - The weighted segment-sum is computed per 128-edge chunk as
    aggr.T += msgs.T @ one_hot  with  one_hot[e, d] =
    (iota[d] == dst_rel[e]) * w[e]  built either per-chunk on the DVE
    (GNN_OH=chunk) or precomputed on the host and streamed from HBM
    (GNN_OH=hbm); PSUM accumulates aggr.T per 128-dst tile.
  - The MLP runs transposed ([channels, nodes]) with stationary weights:
    h1 = relu(W_rel @ [x; aggr] + b), h2 = relu(W_h1 @ h1 + b), out.T =
    W_out @ h2 + b, written back as [3, 12500] per core and transposed on
    the host.
"""

import os

import numpy as np
import ml_dtypes

bf16 = ml_dtypes.bfloat16

N_NODES = 100000
D_IN = 64
D_HID = 128
D_OUT = 3
NC = 8
NPC = N_NODES // NC            # 12500 dst nodes per core
P = 128
PAIR_ROWS = N_NODES // 2       # 50000 pair-packed table rows
N_SEG = 2
SEG_ROWS = PAIR_ROWS // N_SEG  # 25000 rows per segment (int16-addressable)
N_TILES = (NPC + P - 1) // P   # 98 dst tiles per core
MAX_GROUP_CHUNKS = 80          # gather-buffer chunks per group
MLP_BLK = 512

LAST_RESULT = None             # BassKernelResults of the most recent run
_CACHE = {}


def _make_plan(caps):
    """caps: [N_TILES, N_SEG, 2] chunks per (tile, seg, parity) cell.

    Returns group structure; chunk layout within a group is
    (seg major) -> (tile) -> (parity).
    """
    max_gc = int(os.environ.get("GNN_MGC", "56"))
    tiles_chunks = caps.sum(axis=(1, 2))  # chunks per tile
    groups = []
    cur, cur_n = [], 0
    for t in range(N_TILES):
        n = int(tiles_chunks[t])
        if cur and cur_n + n > max_gc:
            groups.append(cur)
            cur, cur_n = [], 0
        cur.append(t)
        cur_n += n
    if cur:
        groups.append(cur)

    plan = []
    c_off = 0
    for tiles in groups:
        g = {"tiles": tiles, "c_off": c_off, "calls": [],
             "tile_chunks": {t: [] for t in tiles}}
        local = 0
        for s in range(N_SEG):
            c0 = local
            for t in tiles:
                for p in range(2):
                    for _ in range(int(caps[t, s, p])):
                        g["tile_chunks"][t].append((local, p))
                        local += 1
            if local > c0:
                g["calls"].append({"s": s, "c0": c0, "n": local - c0})
        g["chunks"] = local
        plan.append(g)
        c_off += local
    return plan, c_off


def _build_nc(caps, plan, C_total):
    from concourse import bacc
    import concourse.mybir as mybir
    import concourse.tile as tile

    dt = mybir.dt
    n_queues = int(os.environ.get("GNN_NQ", "4"))
    nc = bacc.Bacc("TRN2", debug=False, num_swdge_queues=n_queues)

    oh_hbm = os.environ.get("GNN_OH", "chunk") == "hbm"
    table_d = nc.dram_tensor("table", [PAIR_ROWS, P], dt.bfloat16, kind="ExternalInput")
    idx_d = nc.dram_tensor("idx", [P, C_total * 8], dt.int16, kind="ExternalInput")
    if oh_hbm:
        oh_d = nc.dram_tensor("oh", [P, C_total * P], dt.bfloat16, kind="ExternalInput")
    dstrel_d = nc.dram_tensor("dstrel", [P, C_total], dt.float32, kind="ExternalInput")
    wgt_d = nc.dram_tensor("wgt", [P, C_total], dt.float32, kind="ExternalInput")
    xT_d = nc.dram_tensor("xT", [D_IN, NPC], dt.bfloat16, kind="ExternalInput")
    wrx_d = nc.dram_tensor("wrx", [D_IN, D_HID], dt.bfloat16, kind="ExternalInput")
    wra_d = nc.dram_tensor("wra", [D_IN, D_HID], dt.bfloat16, kind="ExternalInput")
    wh1_d = nc.dram_tensor("wh1", [D_HID, D_HID], dt.bfloat16, kind="ExternalInput")
    wout_d = nc.dram_tensor("wout", [D_HID, D_OUT], dt.bfloat16, kind="ExternalInput")
    brel_d = nc.dram_tensor("brel", [D_HID, 1], dt.float32, kind="ExternalInput")
    bh1_d = nc.dram_tensor("bh1", [D_HID, 1], dt.float32, kind="ExternalInput")
    bout_d = nc.dram_tensor("bout", [D_OUT, 1], dt.float32, kind="ExternalInput")
    outT_d = nc.dram_tensor("outT", [D_OUT, NPC], dt.float32, kind="ExternalOutput")

    eq = mybir.AluOpType.is_equal
    mul = mybir.AluOpType.mult
    add = mybir.AluOpType.add
    relu = mybir.ActivationFunctionType.Relu

    _q = [0]

    def _next_queue():
        q = _q[0]
        _q[0] = (q + 1) % n_queues
        return q

    skip_gather = bool(int(os.environ.get("GNN_SKIP_GATHER", "0")))
    skip_agg = bool(int(os.environ.get("GNN_SKIP_AGG", "0")))
    bench_iters = int(os.environ.get("GNN_BENCH_ITERS", "1"))

    with tile.TileContext(nc) as tc:
        with (
            tc.tile_pool(name="const", bufs=1) as co,
            tc.tile_pool(name="gbufp", bufs=int(os.environ.get("GNN_GB", "2"))) as gbufp,
            tc.tile_pool(name="idxp", bufs=2) as idxp,
            tc.tile_pool(name="ohp", bufs=2) as ohp,
            tc.tile_pool(name="outp", bufs=2) as outp,
            tc.tile_pool(name="aggps", bufs=int(os.environ.get("GNN_PS", "4")),
                         space="PSUM") as aggps,
            tc.tile_pool(name="mlpps", bufs=2, space="PSUM") as mlpps,
            tc.tile_pool(name="ops", bufs=2, space="PSUM") as ops,
        ):
            # ---- constants ----
            iota = co.tile([P, P], dt.bfloat16)
            nc.gpsimd.iota(iota[:], pattern=[[1, P]], base=0, channel_multiplier=0,
                           allow_small_or_imprecise_dtypes=True)
            dstrel = co.tile([P, C_total], dt.float32)
            nc.sync.dma_start(dstrel[:], dstrel_d[:])
            wgt = co.tile([P, C_total], dt.float32)
            nc.sync.dma_start(wgt[:], wgt_d[:])
            # negated weights for the ACT-engine one-hot path (one-time)
            negw = co.tile([P, C_total], dt.float32)
            nc.vector.tensor_scalar(negw[:], wgt[:], -1.0, None, mul)
            xT = co.tile([D_IN, NPC], dt.bfloat16)
            nc.sync.dma_start(xT[:], xT_d[:])
            aggrT = co.tile([D_IN, NPC], dt.bfloat16)
            wrx = co.tile([D_IN, D_HID], dt.bfloat16)
            nc.sync.dma_start(wrx[:], wrx_d[:])
            wra = co.tile([D_IN, D_HID], dt.bfloat16)
            nc.sync.dma_start(wra[:], wra_d[:])
            wh1 = co.tile([D_HID, D_HID], dt.bfloat16)
            nc.sync.dma_start(wh1[:], wh1_d[:])
            wout = co.tile([D_HID, D_OUT], dt.bfloat16)
            nc.sync.dma_start(wout[:], wout_d[:])
            brel = co.tile([D_HID, 1], dt.float32)
            nc.sync.dma_start(brel[:], brel_d[:])
            bh1 = co.tile([D_HID, 1], dt.float32)
            nc.sync.dma_start(bh1[:], bh1_d[:])
            bout = co.tile([D_OUT, 1], dt.float32)
            nc.sync.dma_start(bout[:], bout_d[:])
            h1 = co.tile([D_HID, NPC], dt.bfloat16)
            h2 = co.tile([D_HID, NPC], dt.bfloat16)

            no_onehot = bool(int(os.environ.get("GNN_NO_ONEHOT", "0")))
            no_mm = bool(int(os.environ.get("GNN_NO_MM", "0")))
            sp_chunks = int(os.environ.get("GNN_SP", "8"))
            # every GNN_ACTR-th one-hot build goes to the ACT engine
            # (2-op abs/relu form); 0 disables ACT offload
            actr = int(os.environ.get("GNN_ACTR", "0"))
            abs_f = mybir.ActivationFunctionType.Abs

            def emit_agg():
                # aggr.T[f, d] = sum_e w_e * x[src_e, f]
                pending = []   # (psum, t, tw) copies delayed by one group
                ck = [0]       # chunk counter for DVE/ACT round-robin

                def flush_pending():
                    for psum, t, tw in pending:
                        nc.vector.tensor_copy(aggrT[:, t * P : t * P + tw],
                                              psum[:, :tw])
                    pending.clear()

                for g in plan:
                    cg = g["chunks"]
                    o = g["c_off"]
                    gbuf = gbufp.tile([P, cg, P], dt.bfloat16, tag="gb")
                    idxt = idxp.tile([P, cg * 8], dt.int16, tag="ix")
                    nc.sync.dma_start(
                        idxt[:], idx_d[:, o * 8 : (o + cg) * 8])
                    ohg = ohp.tile([P, cg * P], dt.bfloat16, tag="oh")
                    if oh_hbm:
                        nc.sync.dma_start(
                            ohg[:], oh_d[:, o * P : (o + cg) * P])
                    if skip_gather:
                        nc.gpsimd.memset(gbuf[:], 0)
                    else:
                        for call in g["calls"]:
                            s, c0, n = call["s"], call["c0"], call["n"]
                            if sp_chunks <= 0:
                                pieces = [(c0, n)]
                            else:
                                pieces = [
                                    (c0 + p0, min(sp_chunks, n - p0))
                                    for p0 in range(0, n, sp_chunks)
                                ]
                            for pc0, pn in pieces:
                                nc.gpsimd.dma_gather(
                                    out_ap=gbuf[:, pc0 : pc0 + pn, :],
                                    in_ap=table_d[s * SEG_ROWS : (s + 1) * SEG_ROWS, :],
                                    idxs_ap=idxt[:, pc0 * 8 : (pc0 + pn) * 8],
                                    num_idxs=pn * P,
                                    num_idxs_reg=pn * P,
                                    elem_size=P,
                                    # single_packet caps at 64 descs/engine
                                    # (1024 idxs) on HW and crashes beyond
                                    single_packet=sp_chunks > 0,
                                    queue_num=_next_queue(),
                                )
                    # phase A: build all one-hots of this group (no gather
                    # dep — runs while the gather drains)
                    if not (oh_hbm or no_onehot or no_mm or skip_agg):
                        for t in g["tiles"]:
                            for cl, par in g["tile_chunks"][t]:
                                c = o + cl
                                sl = ohg[:, cl * P : (cl + 1) * P]
                                ck[0] += 1
                                if actr > 0 and ck[0] % actr == 0:
                                    # oh = relu(w - w*|dstrel - iota|)
                                    nc.scalar.activation(
                                        sl, iota[:], abs_f,
                                        bias=dstrel[:, c : c + 1],
                                        scale=-1.0)
                                    nc.scalar.activation(
                                        sl, sl, relu,
                                        bias=wgt[:, c : c + 1],
                                        scale=negw[:, c : c + 1])
                                else:
                                    nc.vector.tensor_scalar(
                                        sl, iota[:],
                                        dstrel[:, c : c + 1],
                                        wgt[:, c : c + 1],
                                        eq, mul,
                                    )
                    # copies of the previous group land here: by now their
                    # matmuls have long finished, so the DVE never stalls
                    flush_pending()
                    # phase B: per-tile PSUM accumulation
                    for t in g["tiles"]:
                        clist = g["tile_chunks"][t]
                        tw = min(P, NPC - t * P)
                        if skip_agg or no_mm or not clist:
                            nc.vector.memset(aggrT[:, t * P : t * P + tw], 0)
                            continue
                        psum = aggps.tile([D_IN, P], dt.float32, tag="agg")
                        for i, (cl, par) in enumerate(clist):
                            rhs = (iota[:] if no_onehot
                                   else ohg[:, cl * P : (cl + 1) * P])
                            nc.tensor.matmul(
                                psum[:],
                                gbuf[:, cl, par * D_IN : (par + 1) * D_IN],
                                rhs,
                                start=(i == 0),
                                stop=(i == len(clist) - 1),
                            )
                        pending.append((psum, t, tw))
                flush_pending()

            def emit_mlp():
                nb = (NPC + MLP_BLK - 1) // MLP_BLK
                for b in range(nb):
                    c0 = b * MLP_BLK
                    n = min(MLP_BLK, NPC - c0)
                    ps = mlpps.tile([D_HID, MLP_BLK], dt.float32, tag="mlp")
                    nc.tensor.matmul(ps[:, :n], wrx[:], xT[:, c0 : c0 + n],
                                     start=True, stop=False)
                    nc.tensor.matmul(ps[:, :n], wra[:], aggrT[:, c0 : c0 + n],
                                     start=False, stop=True)
                    nc.scalar.activation(h1[:, c0 : c0 + n], ps[:, :n], relu,
                                         bias=brel[:, 0:1])
                for b in range(nb):
                    c0 = b * MLP_BLK
                    n = min(MLP_BLK, NPC - c0)
                    ps = mlpps.tile([D_HID, MLP_BLK], dt.float32, tag="mlp")
                    nc.tensor.matmul(ps[:, :n], wh1[:], h1[:, c0 : c0 + n],
                                     start=True, stop=True)
                    nc.scalar.activation(h2[:, c0 : c0 + n], ps[:, :n], relu,
                                         bias=bh1[:, 0:1])
                for b in range(nb):
                    c0 = b * MLP_BLK
                    n = min(MLP_BLK, NPC - c0)
                    pso = ops.tile([D_OUT, MLP_BLK], dt.float32, tag="out")
                    nc.tensor.matmul(pso[:, :n], wout[:], h2[:, c0 : c0 + n],
                                     start=True, stop=True)
                    osb = outp.tile([D_OUT, MLP_BLK], dt.float32, tag="osb")
                    nc.vector.tensor_scalar(osb[:, :n], pso[:, :n],
                                            bout[:, 0:1], None, add)
                    nc.sync.dma_start(outT_d[:, c0 : c0 + n], osb[:, :n])

            if bench_iters == 1:
                emit_agg()
                emit_mlp()
            else:
                with tc.For_i(0, bench_iters, 1):
                    emit_agg()
                    emit_mlp()

    nc.compile()
    return nc


def prepare(feature_data, edge_info, edge_weights, W_rel, b_rel, W_h1, b_h1,
            W_out, b_out):
    """Host-side sharding: returns (nc, in_maps)."""
    feature_data = np.asarray(feature_data, dtype=np.float32)
    edge_info = np.asarray(edge_info)
    edge_weights = np.asarray(edge_weights, dtype=np.float32)
    W_rel = np.asarray(W_rel, dtype=np.float32)
    b_rel = np.asarray(b_rel, dtype=np.float32)
    W_h1 = np.asarray(W_h1, dtype=np.float32)
    b_h1 = np.asarray(b_h1, dtype=np.float32)
    W_out = np.asarray(W_out, dtype=np.float32)
    b_out = np.asarray(b_out, dtype=np.float32)

    src = edge_info[0].astype(np.int64)
    dst = edge_info[1].astype(np.int64)
    w = edge_weights
    E = src.shape[0]

    # ---- cell assignment: (core, tile, seg, parity) ----
    core = dst // NPC
    tile_id = (dst % NPC) // P
    pair = src >> 1
    seg = pair // SEG_ROWS
    par = src & 1
    key = (((core * N_TILES) + tile_id) * N_SEG + seg) * 2 + par
    if bool(int(os.environ.get("GNN_SRCSORT", "1"))):
        # src-major within each cell: consecutive gather descriptors hit
        # ascending table rows (HBM row-buffer locality)
        order = np.lexsort((src, key))
    else:
        order = np.argsort(key, kind="stable")
    s_key = key[order]
    n_cells = NC * N_TILES * N_SEG * 2
    counts = np.bincount(s_key, minlength=n_cells)
    starts = np.zeros(n_cells + 1, dtype=np.int64)
    np.cumsum(counts, out=starts[1:])
    counts = counts.reshape(NC, N_TILES, N_SEG, 2)

    caps = (counts.max(axis=0) + P - 1) // P  # [N_TILES, N_SEG, 2] chunks
    plan_key = (caps.tobytes(),) + tuple(
        os.environ.get(k, "0") for k in (
            "GNN_SKIP_GATHER", "GNN_SKIP_AGG", "GNN_BENCH_ITERS",
            "GNN_NO_ONEHOT", "GNN_NO_MM", "GNN_SP", "GNN_NQ", "GNN_OH",
            "GNN_GB", "GNN_MGC", "GNN_ACTR", "GNN_PS"))
    if plan_key in _CACHE:
        nc, plan, C_total = _CACHE[plan_key]
    else:
        plan, C_total = _make_plan(caps)
        nc = _build_nc(caps, plan, C_total)
        _CACHE[plan_key] = (nc, plan, C_total)

    # ---- per-core data in the plan's chunk order ----
    s_idx = (pair - seg * SEG_ROWS)[order].astype(np.int16)
    s_dstrel = ((dst % NPC) % P)[order].astype(np.float32)
    s_w = w[order]

    # slot offset of each cell in the global chunk layout (uniform over cores)
    cell_off = np.zeros((N_TILES, N_SEG, 2), dtype=np.int64)
    for g in plan:
        for call in g["calls"]:
            s = call["s"]
            o = (g["c_off"] + call["c0"]) * P
            for t in g["tiles"]:
                for p in range(2):
                    cell_off[t, s, p] = o
                    o += int(caps[t, s, p]) * P

    table = np.ascontiguousarray(
        feature_data.astype(bf16).reshape(PAIR_ROWS, P))
    wrel = np.ascontiguousarray(W_rel.T).astype(bf16)
    wrx = np.ascontiguousarray(W_rel[:, :D_IN].T).astype(bf16)
    wra = np.ascontiguousarray(W_rel[:, D_IN:].T).astype(bf16)
    wh1 = np.ascontiguousarray(W_h1.T).astype(bf16)
    wout = np.ascontiguousarray(W_out.T).astype(bf16)
    brel = b_rel.reshape(D_HID, 1)
    bh1 = b_h1.reshape(D_HID, 1)
    bout = b_out.reshape(D_OUT, 1)

    in_maps = []
    for c in range(NC):
        idx_flat = np.zeros(C_total * P, dtype=np.int16)
        dr_flat = np.zeros(C_total * P, dtype=np.float32)
        w_flat = np.zeros(C_total * P, dtype=np.float32)
        for t in range(N_TILES):
            for s in range(N_SEG):
                for p in range(2):
                    cell = ((c * N_TILES + t) * N_SEG + s) * 2 + p
                    n = counts[c, t, s, p]
                    if n == 0:
                        continue
                    a = starts[cell]
                    o = cell_off[t, s, p]
                    idx_flat[o : o + n] = s_idx[a : a + n]
                    dr_flat[o : o + n] = s_dstrel[a : a + n]
                    w_flat[o : o + n] = s_w[a : a + n]
        idx_w = np.ascontiguousarray(
            np.tile(idx_flat.reshape(-1, 16).T, (8, 1)))
        dr = np.ascontiguousarray(dr_flat.reshape(C_total, P).T)
        ww = np.ascontiguousarray(w_flat.reshape(C_total, P).T)
        xT = np.ascontiguousarray(
            feature_data[c * NPC : (c + 1) * NPC].T).astype(bf16)
        in_map = {
            "table": table, "idx": idx_w, "dstrel": dr, "wgt": ww, "xT": xT,
            "wrel": wrel, "wrx": wrx, "wra": wra, "wh1": wh1, "wout": wout,
            "brel": brel, "bh1": bh1, "bout": bout,
        }
        if os.environ.get("GNN_OH", "chunk") == "hbm":
            # weighted one-hot rows precomputed on host: oh[e, c*P+d] =
            # w[c*P+e] * (dstrel[c*P+e] == d)
            dr_i = dr_flat.reshape(C_total, P).astype(np.int64)
            ohc = np.zeros((P, C_total, P), dtype=bf16)
            e_ix = np.arange(P)[:, None]
            c_ix = np.arange(C_total)[None, :]
            ohc[e_ix, c_ix, dr_i.T] = w_flat.reshape(C_total, P).T.astype(bf16)
            in_map["oh"] = ohc.reshape(P, C_total * P)
        in_maps.append(in_map)

    return nc, in_maps


def kernel(**inputs):
    global LAST_RESULT
    from concourse.bass_utils import run_bass_kernel_spmd

    nc, in_maps = prepare(**inputs)
    trace = bool(int(os.environ.get("GNN_TRACE", "0")))
    res = run_bass_kernel_spmd(nc, in_maps, core_ids=list(range(NC)),
                               trace=trace)
    LAST_RESULT = res

    out = np.empty((N_NODES, D_OUT), dtype=np.float32)
    for c in range(NC):
        out[c * NPC : (c + 1) * NPC] = res.results[c]["outT"].T
    return out

